# revision 4
# baseline (speedup 1.0000x reference)
"""Multi-head attention (B=2, S=2048, D=1024, H=16) on 8 TRN2 NeuronCores.

Sharding: core c = (batch b = c//4, head-group hg = c%4, 4 heads each).
Tensor-parallel over heads (wq/wk/wv column slices, wo row slices), data
parallel over batch. Host does layout prep (transposes / slices) and the
unshard (concat attn shards, sum the 4 partial outputs per batch).

All matmuls run as float32r (TF32-like, 11-bit mantissa, full PE rate;
the PE datapath rounds operands itself). Softmax skips max-subtraction
(logits ~ N(0,1), no overflow risk); the attention-probability output is
produced by recomputing logits in (q,k) orientation with normalization
fused into the exp as exp(x/8 + ln(1/sum)).
"""

import sys

if "/opt/trn_rl_repo" not in sys.path:
    sys.path.insert(0, "/opt/trn_rl_repo")

import numpy as np

B, S, D = 2, 2048, 1024
NH_LOCAL = 4          # heads per core
DEP = 64              # head depth
HD = NH_LOCAL * DEP   # 256 head-dims per core
N_CORES = 8
QS = 512              # q-stripe width
KT = 128              # k chunk (partition tile)

_CACHE = {}


def _build_program():
    import concourse.bass as bass
    import concourse.mybir as mybir
    import concourse.tile as tile
    from concourse import bacc
    from concourse.masks import make_identity

    F32 = mybir.dt.float32
    F32R = mybir.dt.float32r
    AF = mybir.ActivationFunctionType

    nc = bacc.Bacc(None, target_bir_lowering=False, debug=False)

    qT_d = nc.dram_tensor("qT", [D, S], F32R, kind="ExternalInput")
    kT_d = nc.dram_tensor("kT", [D, S], F32R, kind="ExternalInput")
    vT_d = nc.dram_tensor("vT", [D, S], F32R, kind="ExternalInput")
    wq_d = nc.dram_tensor("wq", [D, HD], F32R, kind="ExternalInput")
    wk_d = nc.dram_tensor("wk", [D, HD], F32R, kind="ExternalInput")
    wv_d = nc.dram_tensor("wv", [D, HD], F32R, kind="ExternalInput")
    bq_d = nc.dram_tensor("bq", [HD, 1], F32, kind="ExternalInput")
    bk_d = nc.dram_tensor("bk", [HD, 1], F32, kind="ExternalInput")
    bv_d = nc.dram_tensor("bv", [1, HD], F32R, kind="ExternalInput")
    wo_d = nc.dram_tensor("wo", [HD, D], F32R, kind="ExternalInput")
    bo_d = nc.dram_tensor("bo", [1, D], F32R, kind="ExternalInput")

    attn_d = nc.dram_tensor("attn_o", [NH_LOCAL, S, S], F32, kind="ExternalOutput")
    out_d = nc.dram_tensor("out_o", [S, D], F32, kind="ExternalOutput")

    n_sq = S // QS      # 4 q stripes
    n_kc = S // KT      # 16 k chunks
    n_dc = D // KT      # 8 contraction chunks for projections

    with tile.TileContext(nc) as tc:
        with (
            tc.tile_pool(name="const", bufs=1) as constp,
            tc.tile_pool(name="wpool", bufs=1) as wpool,
            tc.tile_pool(name="persist", bufs=1) as persist,
            tc.tile_pool(name="xstream", bufs=16) as xstream,
            tc.tile_pool(name="stage", bufs=4) as stage,
            tc.tile_pool(name="ptpool", bufs=4) as ptpool,
            tc.tile_pool(name="eopool", bufs=3) as eopool,
            tc.tile_pool(name="outpool", bufs=4) as outpool,
            tc.tile_pool(name="smalls", bufs=10) as smalls,
            tc.tile_pool(name="psL", bufs=3, space="PSUM") as psL,
            tc.tile_pool(name="psC", bufs=2, space="PSUM") as psC,
            tc.tile_pool(name="psT", bufs=2, space="PSUM") as psT,
        ):
            # ---- constants / weights ----
            ident = constp.tile([128, 128], F32, name="ident")
            make_identity(nc, ident[:])
            ones_f = constp.tile([1, 128], F32, name="ones_f")
            nc.vector.memset(ones_f[:], 1.0)
            ones_r = constp.tile([1, 128], F32R, name="ones_r")
            nc.vector.tensor_copy(ones_r[:], ones_f[:])

            wq_t = [wpool.tile([128, HD], F32R, name=f"wq{i}") for i in range(n_dc)]
            wk_t = [wpool.tile([128, HD], F32R, name=f"wk{i}") for i in range(n_dc)]
            wv_t = [wpool.tile([128, HD], F32R, name=f"wv{i}") for i in range(n_dc)]
            for i in range(n_dc):
                nc.sync.dma_start(wq_t[i][:], wq_d[i * 128:(i + 1) * 128, :])
                nc.sync.dma_start(wk_t[i][:], wk_d[i * 128:(i + 1) * 128, :])
                nc.sync.dma_start(wv_t[i][:], wv_d[i * 128:(i + 1) * 128, :])
            wo_t = [wpool.tile([128, D], F32R, name=f"wo{i}") for i in range(2)]
            for i in range(2):
                nc.sync.dma_start(wo_t[i][:], wo_d[i * 128:(i + 1) * 128, :])
            bq_t = [wpool.tile([128, 1], F32, name=f"bq{i}") for i in range(2)]
            bk_t = [wpool.tile([128, 1], F32, name=f"bk{i}") for i in range(2)]
            for i in range(2):
                nc.sync.dma_start(bq_t[i][:], bq_d[i * 128:(i + 1) * 128, :])
                nc.sync.dma_start(bk_t[i][:], bk_d[i * 128:(i + 1) * 128, :])
            bv_t = wpool.tile([1, HD], F32R, name="bv")
            nc.sync.dma_start(bv_t[:], bv_d[:])
            bo_t = wpool.tile([1, D], F32R, name="bo")
            nc.sync.dma_start(bo_t[:], bo_d[:])

            # ---- persistent activations ----
            qhT = [persist.tile([128, S], F32R, name=f"qhT{m}") for m in range(2)]
            khT = [persist.tile([128, S], F32R, name=f"khT{m}") for m in range(2)]
            # vh_aug: per token-tile, 4 heads x (64 vals + ones col)
            vh = [persist.tile([128, NH_LOCAL * (DEP + 1)], F32R, name=f"vh{m}")
                  for m in range(n_kc)]
            ctxnT = [persist.tile([128, S], F32R, name=f"ctxnT{m}") for m in range(2)]

            ones_col = constp.tile([128, NH_LOCAL], F32, name="ones_col")
            nc.vector.memset(ones_col[:], 1.0)
            for mt in range(n_kc):
                vview = vh[mt].rearrange("p (h c) -> p h c", c=DEP + 1)
                nc.vector.tensor_copy(vview[:, :, DEP], ones_col[:])

            # ---- phase 1: projections ----
            # qhT/khT: (hd, tok) = wX.T @ xT ; vh: (tok, hd) = vT.T @ wv
            for name, xd, wt, bt, ht in (
                ("q", qT_d, wq_t, bq_t, qhT),
                ("k", kT_d, wk_t, bk_t, khT),
            ):
                for n in range(n_sq):
                    xtiles = []
                    for kk in range(n_dc):
                        xt = xstream.tile([128, QS], F32R, tag="xs",
                                          name=f"{name}x{n}_{kk}")
                        nc.sync.dma_start(
                            xt[:], xd[kk * 128:(kk + 1) * 128, n * QS:(n + 1) * QS])
                        xtiles.append(xt)
                    for m in range(2):
                        ps = psL.tile([128, QS], F32, tag="psL", name=f"ps{name}{n}{m}")
                        for kk in range(n_dc):
                            nc.tensor.matmul(
                                ps[:],
                                wt[kk][:, m * 128:(m + 1) * 128],
                                xtiles[kk][:],
                                start=(kk == 0),
                                stop=(kk == n_dc - 1),
                            )
                        nc.vector.tensor_scalar_add(
                            ht[m][:, n * QS:(n + 1) * QS], ps[:], bt[m][:])

            for n in range(n_sq):
                xtiles = []
                for kk in range(n_dc):
                    xt = xstream.tile([128, QS], F32R, tag="xs", name=f"vx{n}_{kk}")
                    nc.sync.dma_start(
                        xt[:], vT_d[kk * 128:(kk + 1) * 128, n * QS:(n + 1) * QS])
                    xtiles.append(xt)
                for j in range(QS // 128):
                    mt = n * (QS // 128) + j
                    ps = psL.tile([128, HD], F32, tag="psL", name=f"psv{mt}")
                    for kk in range(n_dc):
                        nc.tensor.matmul(
                            ps[:],
                            xtiles[kk][:, j * 128:(j + 1) * 128],
                            wv_t[kk][:],
                            start=(kk == 0),
                            stop=False,
                        )
                    nc.tensor.matmul(ps[:], ones_r[:], bv_t[:], start=False, stop=True)
                    vview = vh[mt].rearrange("p (h c) -> p h c", c=DEP + 1)
                    psview = ps.rearrange("p (h c) -> p h c", c=DEP)
                    nc.vector.tensor_copy(vview[:, :, 0:DEP], psview[:])

            # ---- phase 2: attention ----
            lnr_tiles = {}
            for qs in range(n_sq):
                for h in range(NH_LOCAL):
                    ti, r0 = h // 2, (h % 2) * DEP
                    k_rows = khT[ti][r0:r0 + DEP, :]
                    q_rows = qhT[ti][r0:r0 + DEP, :]

                    # pass A: P^T stripes + ctx/sums accumulation
                    pc = psC.tile([DEP + 1, QS], F32, tag="psC", name=f"pc{qs}{h}")
                    for kt in range(n_kc):
                        pl = psL.tile([128, QS], F32, tag="psL", name=f"pl{qs}{h}{kt}")
                        nc.tensor.matmul(
                            pl[:],
                            k_rows[:, kt * 128:(kt + 1) * 128],
                            q_rows[:, qs * QS:(qs + 1) * QS],
                            start=True, stop=True,
                        )
                        pt = ptpool.tile([128, QS], F32R, tag="pt",
                                         name=f"pt{qs}{h}{kt}")
                        nc.scalar.activation(
                            pt[:], pl[:], AF.Exp, bias=0.0, scale=0.125)
                        nc.tensor.matmul(
                            pc[:],
                            vh[kt][:, h * (DEP + 1):(h + 1) * (DEP + 1)],
                            pt[:],
                            start=(kt == 0),
                            stop=(kt == n_kc - 1),
                        )

                    ctxT = stage.tile([DEP + 1, QS], F32, tag="ctxT",
                                      name=f"ctxT{qs}{h}")
                    nc.vector.tensor_copy(ctxT[:], pc[:])
                    for j in range(QS // 128):
                        pt1 = psT.tile([128, DEP + 1], F32, tag="psT",
                                       name=f"pt1_{qs}{h}{j}")
                        nc.tensor.transpose(
                            pt1[:], ctxT[:, j * 128:(j + 1) * 128],
                            ident[0:DEP + 1, 0:DEP + 1])
                        rcp = smalls.tile([128, 1], F32, tag="rcp",
                                          name=f"rcp{qs}{h}{j}")
                        nc.vector.reciprocal(rcp[:], pt1[:, DEP:DEP + 1])
                        lnr = smalls.tile([128, 1], F32, tag="lnr",
                                          name=f"lnr{qs}{h}{j}")
                        nc.scalar.activation(lnr[:], rcp[:], AF.Ln,
                                             bias=0.0, scale=1.0)
                        lnr_tiles[(qs, h, j)] = lnr
                        ctxn = smalls.tile([128, DEP], F32, tag="ctxn",
                                           name=f"ctxn{qs}{h}{j}")
                        nc.vector.tensor_scalar_mul(ctxn[:], pt1[:, 0:DEP], rcp[:])
                        pt2 = psT.tile([DEP, 128], F32, tag="psT",
                                       name=f"pt2_{qs}{h}{j}")
                        nc.tensor.transpose(pt2[:], ctxn[:], ident[:])
                        nc.vector.tensor_copy(
                            ctxnT[ti][r0:r0 + DEP,
                                      qs * QS + j * 128:qs * QS + (j + 1) * 128],
                            pt2[:])

                    # pass B: attn output, normalization fused into exp bias
                    for j in range(QS // 128):
                        q0 = qs * QS + j * 128
                        eo = eopool.tile([128, S], F32, tag="eo",
                                         name=f"eo{qs}{h}{j}")
                        for ks in range(n_sq):
                            pb = psL.tile([128, QS], F32, tag="psL",
                                          name=f"pb{qs}{h}{j}{ks}")
                            nc.tensor.matmul(
                                pb[:],
                                q_rows[:, q0:q0 + 128],
                                k_rows[:, ks * QS:(ks + 1) * QS],
                                start=True, stop=True,
                            )
                            nc.scalar.activation(
                                eo[:, ks * QS:(ks + 1) * QS], pb[:], AF.Exp,
                                bias=lnr_tiles[(qs, h, j)][:], scale=0.125)
                        nc.sync.dma_start(attn_d[h, q0:q0 + 128, :], eo[:])

                # ---- phase 3: output projection for this q-stripe ----
                for j in range(QS // 128):
                    q0 = qs * QS + j * 128
                    for n in range(2):
                        po = psL.tile([128, 512], F32, tag="psL",
                                      name=f"po{qs}{j}{n}")
                        nc.tensor.matmul(
                            po[:], ctxnT[0][:, q0:q0 + 128],
                            wo_t[0][:, n * 512:(n + 1) * 512],
                            start=True, stop=False)
                        nc.tensor.matmul(
                            po[:], ctxnT[1][:, q0:q0 + 128],
                            wo_t[1][:, n * 512:(n + 1) * 512],
                            start=False, stop=False)
                        nc.tensor.matmul(
                            po[:], ones_r[:], bo_t[:, n * 512:(n + 1) * 512],
                            start=False, stop=True)
                        outst = outpool.tile([128, 512], F32, tag="outst",
                                             name=f"outst{qs}{j}{n}")
                        nc.vector.tensor_copy(outst[:], po[:])
                        nc.sync.dma_start(
                            out_d[q0:q0 + 128, n * 512:(n + 1) * 512], outst[:])

    nc.finalize()
    return nc


def _get_runner():
    if "runner" in _CACHE:
        return _CACHE["runner"]

    import jax
    import jax.numpy as jnp
    from jax.sharding import Mesh, PartitionSpec, NamedSharding
    from jax.experimental.shard_map import shard_map
    import concourse.mybir as mybir
    from concourse import bass2jax

    nc = _build_program()
    bass2jax.install_neuronx_cc_hook()

    partition_name = (nc.partition_id_tensor.name
                      if nc.partition_id_tensor else None)
    in_names, out_names, out_avals = [], [], []
    for alloc in nc.m.functions[0].allocations:
        if not isinstance(alloc, mybir.MemoryLocationSet):
            continue
        name = alloc.memorylocations[0].name
        if alloc.kind == "ExternalInput":
            if name != partition_name:
                in_names.append(name)
        elif alloc.kind == "ExternalOutput":
            out_names.append(name)
            out_avals.append(
                jax.core.ShapedArray(tuple(alloc.tensor_shape),
                                     mybir.dt.np(alloc.dtype)))
    n_params = len(in_names)
    n_outs = len(out_names)
    all_in_names = in_names + out_names
    if partition_name is not None:
        all_in_names = all_in_names + [partition_name]

    def _body(*args):
        operands = list(args)
        if partition_name is not None:
            operands.append(bass2jax.partition_id_tensor())
        outs = bass2jax._bass_exec_p.bind(
            *operands,
            out_avals=tuple(out_avals),
            in_names=tuple(all_in_names),
            out_names=tuple(out_names),
            lowering_input_output_aliases=(),
            sim_require_finite=True,
            sim_require_nnan=True,
            nc=nc,
        )
        return tuple(outs)

    devices = jax.devices()[:N_CORES]
    mesh = Mesh(np.asarray(devices), ("core",))
    spec = NamedSharding(mesh, PartitionSpec("core"))
    donate = tuple(range(n_params, n_params + n_outs))
    sharded = jax.jit(
        shard_map(
            _body, mesh=mesh,
            in_specs=(PartitionSpec("core"),) * (n_params + n_outs),
            out_specs=(PartitionSpec("core"),) * n_outs,
            check_rep=False,
        ),
        donate_argnums=donate,
        keep_unused=True,
    )

    zero_shapes = [(N_CORES * a.shape[0], *a.shape[1:]) for a in out_avals]
    zero_dtypes = [a.dtype for a in out_avals]

    def _make_zeros():
        mk = jax.jit(
            lambda: tuple(
                jnp.zeros(s, d) for s, d in zip(zero_shapes, zero_dtypes)),
            out_shardings=(spec,) * n_outs)
        return mk()

    runner = {
        "nc": nc,
        "in_names": in_names,
        "out_names": out_names,
        "out_avals": out_avals,
        "sharded": sharded,
        "spec": spec,
        "make_zeros": _make_zeros,
    }
    _CACHE["runner"] = runner
    return runner


def _prep_core_inputs(v, k, q, wq, bq, wk, bk, wv, bv, wo, bo):
    """Returns per-core dict of device input arrays."""
    f = np.float32
    qT = [np.ascontiguousarray(q[b].T, dtype=f) for b in range(B)]
    kT = [np.ascontiguousarray(k[b].T, dtype=f) for b in range(B)]
    vT = [np.ascontiguousarray(v[b].T, dtype=f) for b in range(B)]
    maps = []
    for c in range(N_CORES):
        b, hg = c // 4, c % 4
        hs = slice(hg * HD, (hg + 1) * HD)
        maps.append({
            "qT": qT[b], "kT": kT[b], "vT": vT[b],
            "wq": np.ascontiguousarray(wq[:, hs], dtype=f),
            "wk": np.ascontiguousarray(wk[:, hs], dtype=f),
            "wv": np.ascontiguousarray(wv[:, hs], dtype=f),
            "bq": np.ascontiguousarray(bq[hs].reshape(HD, 1), dtype=f),
            "bk": np.ascontiguousarray(bk[hs].reshape(HD, 1), dtype=f),
            "bv": np.ascontiguousarray(bv[hs].reshape(1, HD), dtype=f),
            "wo": np.ascontiguousarray(wo[hs, :], dtype=f),
            "bo": (np.ascontiguousarray(bo.reshape(1, D), dtype=f)
                   if hg == 0 else np.zeros((1, D), dtype=f)),
        })
    return maps


def _run_device(maps, runner, reps=1):
    import jax

    concat = [
        jax.device_put(
            np.concatenate([maps[c][name] for c in range(N_CORES)], axis=0),
            runner["spec"])
        for name in runner["in_names"]
    ]
    out = None
    for _ in range(reps):
        zeros = runner["make_zeros"]()
        out = runner["sharded"](*concat, *zeros)
    return out


def benchmark(inputs, reps=20):
    """Times device execution (inputs resident, outputs ping-pong donated).

    Returns list of per-rep wall seconds.
    """
    import time
    import jax

    inputs = {k: np.asarray(v) for k, v in inputs.items()}
    runner = _get_runner()
    maps = _prep_core_inputs(**inputs)
    concat = [
        jax.device_put(
            np.concatenate([maps[c][name] for c in range(N_CORES)], axis=0),
            runner["spec"])
        for name in runner["in_names"]
    ]
    outs = runner["sharded"](*concat, *runner["make_zeros"]())
    jax.block_until_ready(outs)
    times = []
    for _ in range(reps):
        t0 = time.perf_counter()
        outs = runner["sharded"](*concat, *outs)
        jax.block_until_ready(outs)
        times.append(time.perf_counter() - t0)
    return times


def kernel(**inputs):
    inputs = {k: np.asarray(v) for k, v in inputs.items()}
    runner = _get_runner()
    maps = _prep_core_inputs(**inputs)
    out_arrs = _run_device(maps, runner)

    res = {name: np.asarray(a) for name, a in zip(runner["out_names"], out_arrs)}
    attn_all = res["attn_o"].reshape(N_CORES, NH_LOCAL, S, S)
    out_all = res["out_o"].reshape(N_CORES, S, D)

    attn = np.empty((B, 16, S, S), dtype=np.float32)
    out = np.zeros((B, S, D), dtype=np.float32)
    for c in range(N_CORES):
        b, hg = c // 4, c % 4
        attn[b, hg * NH_LOCAL:(hg + 1) * NH_LOCAL] = attn_all[c]
        out[b] += out_all[c]
    return out, attn


if __name__ == "__main__":
    rng = np.random.default_rng(0)
    sc = 1.0 / np.sqrt(D)
    ins = dict(
        v=rng.standard_normal((B, S, D), dtype=np.float32),
        k=rng.standard_normal((B, S, D), dtype=np.float32),
        q=rng.standard_normal((B, S, D), dtype=np.float32),
        wq=(rng.standard_normal((D, D)) * sc).astype(np.float32),
        bq=np.zeros(D, np.float32),
        wk=(rng.standard_normal((D, D)) * sc).astype(np.float32),
        bk=np.zeros(D, np.float32),
        wv=(rng.standard_normal((D, D)) * sc).astype(np.float32),
        bv=np.zeros(D, np.float32),
        wo=(rng.standard_normal((D, D)) * sc).astype(np.float32),
        bo=np.zeros(D, np.float32),
    )
    out, attn = kernel(**ins)
    print("out", out.shape, "attn", attn.shape)
    # quick numpy reference on a slice
    qh = (ins["q"][0] @ ins["wq"] + ins["bq"])[:, :64]
    kh = (ins["k"][0] @ ins["wk"] + ins["bk"])[:, :64]
    lg = qh @ kh.T / 8.0
    a0 = np.exp(lg - lg.max(1, keepdims=True))
    a0 /= a0.sum(1, keepdims=True)
    err = np.abs(attn[0, 0] - a0).max() / np.abs(a0).max()
    print("attn[0,0] relerr vs numpy:", err)


# revision 6
# speedup vs baseline: 1.2535x; 1.2535x over previous
"""Multi-head attention (B=2, S=2048, D=1024, H=16) on 8 TRN2 NeuronCores.

Sharding: core c = (batch b = c//4, head-group hg = c%4, 4 heads each).
Tensor-parallel over heads (wq/wk/wv column slices, wo row slices), data
parallel over batch. Host does layout prep (transposes / slices) and the
unshard (concat attn shards, sum the 4 partial outputs per batch).

All matmuls run as float32r (TF32-like, 11-bit mantissa, full PE rate;
the PE datapath rounds operands itself). Softmax skips max-subtraction
(logits ~ N(0,1), no overflow risk); the attention-probability output is
produced by recomputing logits in (q,k) orientation with normalization
fused into the exp as exp(x/8 + ln(1/sum)).
"""

import sys

if "/opt/trn_rl_repo" not in sys.path:
    sys.path.insert(0, "/opt/trn_rl_repo")

import numpy as np

B, S, D = 2, 2048, 1024
NH_LOCAL = 4          # heads per core
DEP = 64              # head depth
HD = NH_LOCAL * DEP   # 256 head-dims per core
N_CORES = 8
QS = 512              # q-stripe width
KT = 128              # k chunk (partition tile)

_CACHE = {}


def _build_program():
    import concourse.bass as bass
    import concourse.mybir as mybir
    import concourse.tile as tile
    from concourse import bacc
    from concourse.masks import make_identity

    F32 = mybir.dt.float32
    F32R = mybir.dt.float32r
    AF = mybir.ActivationFunctionType

    nc = bacc.Bacc(None, target_bir_lowering=False, debug=False)

    qT_d = nc.dram_tensor("qT", [D, S], F32R, kind="ExternalInput")
    kT_d = nc.dram_tensor("kT", [D, S], F32R, kind="ExternalInput")
    vT_d = nc.dram_tensor("vT", [D, S], F32R, kind="ExternalInput")
    wq_d = nc.dram_tensor("wq", [D, HD], F32R, kind="ExternalInput")
    wk_d = nc.dram_tensor("wk", [D, HD], F32R, kind="ExternalInput")
    wv_d = nc.dram_tensor("wv", [D, HD], F32R, kind="ExternalInput")
    bq_d = nc.dram_tensor("bq", [HD, 1], F32, kind="ExternalInput")
    bk_d = nc.dram_tensor("bk", [HD, 1], F32, kind="ExternalInput")
    bv_d = nc.dram_tensor("bv", [1, HD], F32R, kind="ExternalInput")
    wo_d = nc.dram_tensor("wo", [HD, D], F32R, kind="ExternalInput")
    bo_d = nc.dram_tensor("bo", [1, D], F32R, kind="ExternalInput")

    attn_d = nc.dram_tensor("attn_o", [NH_LOCAL, S, S], F32, kind="ExternalOutput")
    out_d = nc.dram_tensor("out_o", [S, D], F32, kind="ExternalOutput")

    n_sq = S // QS      # 4 q stripes
    n_kc = S // KT      # 16 k chunks
    n_dc = D // KT      # 8 contraction chunks for projections

    with tile.TileContext(nc) as tc:
        with (
            tc.tile_pool(name="const", bufs=1) as constp,
            tc.tile_pool(name="wpool", bufs=1) as wpool,
            tc.tile_pool(name="persist", bufs=1) as persist,
            tc.tile_pool(name="xstream", bufs=8) as xstream,
            tc.tile_pool(name="stage", bufs=4) as stage,
            tc.tile_pool(name="ptpool", bufs=10) as ptpool,
            tc.tile_pool(name="eopool", bufs=3) as eopool,
            tc.tile_pool(name="outpool", bufs=3) as outpool,
            tc.tile_pool(name="smalls", bufs=12) as smalls,
            tc.tile_pool(name="psL", bufs=3, space="PSUM") as psL,
            tc.tile_pool(name="psS", bufs=2, space="PSUM") as psS,
        ):
            # ---- constants / weights ----
            ident = constp.tile([128, 128], F32, name="ident")
            make_identity(nc, ident[:])
            ones_f = constp.tile([1, 128], F32, name="ones_f")
            nc.vector.memset(ones_f[:], 1.0)
            ones_r = constp.tile([1, 128], F32R, name="ones_r")
            nc.vector.tensor_copy(ones_r[:], ones_f[:])

            wq_t = [wpool.tile([128, HD], F32R, name=f"wq{i}") for i in range(n_dc)]
            wk_t = [wpool.tile([128, HD], F32R, name=f"wk{i}") for i in range(n_dc)]
            wv_t = [wpool.tile([128, HD], F32R, name=f"wv{i}") for i in range(n_dc)]
            for i in range(n_dc):
                nc.sync.dma_start(wq_t[i][:], wq_d[i * 128:(i + 1) * 128, :])
                nc.sync.dma_start(wk_t[i][:], wk_d[i * 128:(i + 1) * 128, :])
                nc.sync.dma_start(wv_t[i][:], wv_d[i * 128:(i + 1) * 128, :])
            wo_t = [wpool.tile([128, D], F32R, name=f"wo{i}") for i in range(2)]
            for i in range(2):
                nc.sync.dma_start(wo_t[i][:], wo_d[i * 128:(i + 1) * 128, :])
            bq_t = [wpool.tile([128, 1], F32, name=f"bq{i}") for i in range(2)]
            bk_t = [wpool.tile([128, 1], F32, name=f"bk{i}") for i in range(2)]
            for i in range(2):
                nc.sync.dma_start(bq_t[i][:], bq_d[i * 128:(i + 1) * 128, :])
                nc.sync.dma_start(bk_t[i][:], bk_d[i * 128:(i + 1) * 128, :])
            bv_t = wpool.tile([1, HD], F32R, name="bv")
            nc.sync.dma_start(bv_t[:], bv_d[:])
            bo_t = wpool.tile([1, D], F32R, name="bo")
            nc.sync.dma_start(bo_t[:], bo_d[:])

            # ---- persistent activations ----
            qhT = [persist.tile([128, S], F32R, name=f"qhT{m}") for m in range(2)]
            khT = [persist.tile([128, S], F32R, name=f"khT{m}") for m in range(2)]
            # vh_aug: per token-tile, 4 heads x (64 vals + ones col)
            vh = [persist.tile([128, NH_LOCAL * (DEP + 1)], F32R, name=f"vh{m}")
                  for m in range(n_kc)]
            ctxnT = [persist.tile([128, S], F32R, name=f"ctxnT{m}") for m in range(2)]

            ones_col = constp.tile([128, NH_LOCAL], F32, name="ones_col")
            nc.vector.memset(ones_col[:], 1.0)
            for mt in range(n_kc):
                vview = vh[mt].rearrange("p (h c) -> p h c", c=DEP + 1)
                nc.vector.tensor_copy(vview[:, :, DEP], ones_col[:])

            # ---- phase 1: projections ----
            # qhT/khT: (hd, tok) = wX.T @ xT ; vh: (tok, hd) = vT.T @ wv
            for name, xd, wt, bt, ht in (
                ("q", qT_d, wq_t, bq_t, qhT),
                ("k", kT_d, wk_t, bk_t, khT),
            ):
                for n in range(n_sq):
                    xtiles = []
                    for kk in range(n_dc):
                        xt = xstream.tile([128, QS], F32R, tag="xs",
                                          name=f"{name}x{n}_{kk}")
                        nc.sync.dma_start(
                            xt[:], xd[kk * 128:(kk + 1) * 128, n * QS:(n + 1) * QS])
                        xtiles.append(xt)
                    for m in range(2):
                        ps = psL.tile([128, QS], F32, tag="psL", name=f"ps{name}{n}{m}")
                        for kk in range(n_dc):
                            nc.tensor.matmul(
                                ps[:],
                                wt[kk][:, m * 128:(m + 1) * 128],
                                xtiles[kk][:],
                                start=(kk == 0),
                                stop=(kk == n_dc - 1),
                            )
                        nc.vector.tensor_scalar_add(
                            ht[m][:, n * QS:(n + 1) * QS], ps[:], bt[m][:])

            for n in range(n_sq):
                xtiles = []
                for kk in range(n_dc):
                    xt = xstream.tile([128, QS], F32R, tag="xs", name=f"vx{n}_{kk}")
                    nc.sync.dma_start(
                        xt[:], vT_d[kk * 128:(kk + 1) * 128, n * QS:(n + 1) * QS])
                    xtiles.append(xt)
                for j in range(QS // 128):
                    mt = n * (QS // 128) + j
                    ps = psL.tile([128, HD], F32, tag="psL", name=f"psv{mt}")
                    for kk in range(n_dc):
                        nc.tensor.matmul(
                            ps[:],
                            xtiles[kk][:, j * 128:(j + 1) * 128],
                            wv_t[kk][:],
                            start=(kk == 0),
                            stop=False,
                        )
                    nc.tensor.matmul(ps[:], ones_r[:], bv_t[:], start=False, stop=True)
                    vview = vh[mt].rearrange("p (h c) -> p h c", c=DEP + 1)
                    psview = ps.rearrange("p (h c) -> p h c", c=DEP)
                    nc.vector.tensor_copy(vview[:, :, 0:DEP], psview[:])

            # ---- phase 2: attention ----
            # Per stripe: pass A (dense logits burst + exp pairs + attnV
            # burst per head), then batched Ln (one table switch), then
            # ctx-normalize transposes, then pass B (attn output with
            # normalization fused into the exp bias), then out-proj.
            lnr_tiles = {}
            rcp_tiles = {}
            for qs in range(n_sq):
                pt_tiles = {}
                for h in range(NH_LOCAL):
                    ti, r0 = h // 2, (h % 2) * DEP
                    k_rows = khT[ti][r0:r0 + DEP, :]
                    q_rows = qhT[ti][r0:r0 + DEP, :]

                    # pass A: logits in pairs (one exp per 2 PSUM banks)
                    for kp in range(n_kc // 2):
                        pl = psL.tile([128, 2 * QS], F32, tag="psL",
                                      name=f"pl{qs}{h}{kp}")
                        for half in range(2):
                            kt = kp * 2 + half
                            nc.tensor.matmul(
                                pl[:, half * QS:(half + 1) * QS],
                                k_rows[:, kt * 128:(kt + 1) * 128],
                                q_rows[:, qs * QS:(qs + 1) * QS],
                                start=True, stop=True,
                            )
                        pt = ptpool.tile([128, 2 * QS], F32R, tag="pt",
                                         name=f"pt{qs}{h}{kp}")
                        nc.scalar.activation(
                            pt[:], pl[:], AF.Exp, bias=0.0, scale=0.125)
                        pt_tiles[(h, kp)] = pt

                    # attnV burst: ctx_aug^T accumulation over all k chunks
                    pc = psS.tile([DEP + 1, QS], F32, tag="psS", name=f"pc{qs}{h}")
                    for kt in range(n_kc):
                        pt = pt_tiles[(h, kt // 2)]
                        nc.tensor.matmul(
                            pc[:],
                            vh[kt][:, h * (DEP + 1):(h + 1) * (DEP + 1)],
                            pt[:, (kt % 2) * QS:(kt % 2 + 1) * QS],
                            start=(kt == 0),
                            stop=(kt == n_kc - 1),
                        )

                    ctxT = stage.tile([DEP + 1, QS], F32, tag="ctxT",
                                      name=f"ctxT{qs}{h}")
                    nc.vector.tensor_copy(ctxT[:], pc[:])
                    for j in range(QS // 128):
                        pt1 = psS.tile([128, DEP + 1], F32, tag="psS",
                                       name=f"pt1_{qs}{h}{j}")
                        nc.tensor.transpose(
                            pt1[:], ctxT[:, j * 128:(j + 1) * 128],
                            ident[0:DEP + 1, 0:DEP + 1])
                        rcp = smalls.tile([128, 1], F32, tag="rcp",
                                          name=f"rcp{qs}{h}{j}")
                        nc.vector.reciprocal(rcp[:], pt1[:, DEP:DEP + 1])
                        rcp_tiles[(qs, h, j)] = rcp
                        ctxn = smalls.tile([128, DEP], F32, tag="ctxn",
                                           name=f"ctxn{qs}{h}{j}")
                        nc.vector.tensor_scalar_mul(ctxn[:], pt1[:, 0:DEP], rcp[:])
                        pt2 = psS.tile([DEP, 128], F32, tag="psS",
                                       name=f"pt2_{qs}{h}{j}")
                        nc.tensor.transpose(pt2[:], ctxn[:], ident[:])
                        nc.vector.tensor_copy(
                            ctxnT[ti][r0:r0 + DEP,
                                      qs * QS + j * 128:qs * QS + (j + 1) * 128],
                            pt2[:])

                # batched Ln for the whole stripe (2 table switches total)
                for h in range(NH_LOCAL):
                    for j in range(QS // 128):
                        lnr = smalls.tile([128, 1], F32, tag="lnr",
                                          name=f"lnr{qs}{h}{j}")
                        nc.scalar.activation(lnr[:], rcp_tiles[(qs, h, j)],
                                             AF.Ln, bias=0.0, scale=1.0)
                        lnr_tiles[(qs, h, j)] = lnr

                # pass B: attn output (normalization fused into exp bias)
                for h in range(NH_LOCAL):
                    ti, r0 = h // 2, (h % 2) * DEP
                    k_rows = khT[ti][r0:r0 + DEP, :]
                    q_rows = qhT[ti][r0:r0 + DEP, :]
                    for j in range(QS // 128):
                        q0 = qs * QS + j * 128
                        eo = eopool.tile([128, S], F32, tag="eo",
                                         name=f"eo{qs}{h}{j}")
                        for ksp in range(2):
                            pb = psL.tile([128, 2 * QS], F32, tag="psL",
                                          name=f"pb{qs}{h}{j}{ksp}")
                            for half in range(2):
                                ks = ksp * 2 + half
                                nc.tensor.matmul(
                                    pb[:, half * QS:(half + 1) * QS],
                                    q_rows[:, q0:q0 + 128],
                                    k_rows[:, ks * QS:(ks + 1) * QS],
                                    start=True, stop=True,
                                )
                            nc.scalar.activation(
                                eo[:, ksp * 2 * QS:(ksp + 1) * 2 * QS], pb[:],
                                AF.Exp, bias=lnr_tiles[(qs, h, j)][:], scale=0.125)
                        nc.sync.dma_start(attn_d[h, q0:q0 + 128, :], eo[:])

                # ---- phase 3: output projection for this q-stripe ----
                for j in range(QS // 128):
                    q0 = qs * QS + j * 128
                    po = psL.tile([128, 2 * QS], F32, tag="psL", name=f"po{qs}{j}")
                    for n in range(2):
                        sl = po[:, n * QS:(n + 1) * QS]
                        nc.tensor.matmul(
                            sl, ctxnT[0][:, q0:q0 + 128],
                            wo_t[0][:, n * 512:(n + 1) * 512],
                            start=True, stop=False)
                        nc.tensor.matmul(
                            sl, ctxnT[1][:, q0:q0 + 128],
                            wo_t[1][:, n * 512:(n + 1) * 512],
                            start=False, stop=False)
                        nc.tensor.matmul(
                            sl, ones_r[:], bo_t[:, n * 512:(n + 1) * 512],
                            start=False, stop=True)
                    outst = outpool.tile([128, 2 * QS], F32, tag="outst",
                                         name=f"outst{qs}{j}")
                    nc.vector.tensor_copy(outst[:], po[:])
                    nc.sync.dma_start(out_d[q0:q0 + 128, :], outst[:])

    nc.finalize()
    return nc


def _get_runner():
    if "runner" in _CACHE:
        return _CACHE["runner"]

    import jax
    import jax.numpy as jnp
    from jax.sharding import Mesh, PartitionSpec, NamedSharding
    from jax.experimental.shard_map import shard_map
    import concourse.mybir as mybir
    from concourse import bass2jax

    nc = _build_program()
    bass2jax.install_neuronx_cc_hook()

    partition_name = (nc.partition_id_tensor.name
                      if nc.partition_id_tensor else None)
    in_names, out_names, out_avals = [], [], []
    for alloc in nc.m.functions[0].allocations:
        if not isinstance(alloc, mybir.MemoryLocationSet):
            continue
        name = alloc.memorylocations[0].name
        if alloc.kind == "ExternalInput":
            if name != partition_name:
                in_names.append(name)
        elif alloc.kind == "ExternalOutput":
            out_names.append(name)
            out_avals.append(
                jax.core.ShapedArray(tuple(alloc.tensor_shape),
                                     mybir.dt.np(alloc.dtype)))
    n_params = len(in_names)
    n_outs = len(out_names)
    all_in_names = in_names + out_names
    if partition_name is not None:
        all_in_names = all_in_names + [partition_name]

    def _body(*args):
        operands = list(args)
        if partition_name is not None:
            operands.append(bass2jax.partition_id_tensor())
        outs = bass2jax._bass_exec_p.bind(
            *operands,
            out_avals=tuple(out_avals),
            in_names=tuple(all_in_names),
            out_names=tuple(out_names),
            lowering_input_output_aliases=(),
            sim_require_finite=True,
            sim_require_nnan=True,
            nc=nc,
        )
        return tuple(outs)

    devices = jax.devices()[:N_CORES]
    mesh = Mesh(np.asarray(devices), ("core",))
    spec = NamedSharding(mesh, PartitionSpec("core"))
    donate = tuple(range(n_params, n_params + n_outs))
    sharded = jax.jit(
        shard_map(
            _body, mesh=mesh,
            in_specs=(PartitionSpec("core"),) * (n_params + n_outs),
            out_specs=(PartitionSpec("core"),) * n_outs,
            check_rep=False,
        ),
        donate_argnums=donate,
        keep_unused=True,
    )

    zero_shapes = [(N_CORES * a.shape[0], *a.shape[1:]) for a in out_avals]
    zero_dtypes = [a.dtype for a in out_avals]

    def _make_zeros():
        mk = jax.jit(
            lambda: tuple(
                jnp.zeros(s, d) for s, d in zip(zero_shapes, zero_dtypes)),
            out_shardings=(spec,) * n_outs)
        return mk()

    runner = {
        "nc": nc,
        "in_names": in_names,
        "out_names": out_names,
        "out_avals": out_avals,
        "sharded": sharded,
        "spec": spec,
        "make_zeros": _make_zeros,
    }
    _CACHE["runner"] = runner
    return runner


def _prep_core_inputs(v, k, q, wq, bq, wk, bk, wv, bv, wo, bo):
    """Returns per-core dict of device input arrays."""
    f = np.float32
    qT = [np.ascontiguousarray(q[b].T, dtype=f) for b in range(B)]
    kT = [np.ascontiguousarray(k[b].T, dtype=f) for b in range(B)]
    vT = [np.ascontiguousarray(v[b].T, dtype=f) for b in range(B)]
    maps = []
    for c in range(N_CORES):
        b, hg = c // 4, c % 4
        hs = slice(hg * HD, (hg + 1) * HD)
        maps.append({
            "qT": qT[b], "kT": kT[b], "vT": vT[b],
            "wq": np.ascontiguousarray(wq[:, hs], dtype=f),
            "wk": np.ascontiguousarray(wk[:, hs], dtype=f),
            "wv": np.ascontiguousarray(wv[:, hs], dtype=f),
            "bq": np.ascontiguousarray(bq[hs].reshape(HD, 1), dtype=f),
            "bk": np.ascontiguousarray(bk[hs].reshape(HD, 1), dtype=f),
            "bv": np.ascontiguousarray(bv[hs].reshape(1, HD), dtype=f),
            "wo": np.ascontiguousarray(wo[hs, :], dtype=f),
            "bo": (np.ascontiguousarray(bo.reshape(1, D), dtype=f)
                   if hg == 0 else np.zeros((1, D), dtype=f)),
        })
    return maps


def _run_device(maps, runner, reps=1):
    import jax

    concat = [
        jax.device_put(
            np.concatenate([maps[c][name] for c in range(N_CORES)], axis=0),
            runner["spec"])
        for name in runner["in_names"]
    ]
    out = None
    for _ in range(reps):
        zeros = runner["make_zeros"]()
        out = runner["sharded"](*concat, *zeros)
    return out


def benchmark(inputs, reps=20):
    """Times device execution (inputs resident, outputs ping-pong donated).

    Returns list of per-rep wall seconds.
    """
    import time
    import jax

    inputs = {k: np.asarray(v) for k, v in inputs.items()}
    runner = _get_runner()
    maps = _prep_core_inputs(**inputs)
    concat = [
        jax.device_put(
            np.concatenate([maps[c][name] for c in range(N_CORES)], axis=0),
            runner["spec"])
        for name in runner["in_names"]
    ]
    outs = runner["sharded"](*concat, *runner["make_zeros"]())
    jax.block_until_ready(outs)
    times = []
    for _ in range(reps):
        t0 = time.perf_counter()
        outs = runner["sharded"](*concat, *outs)
        jax.block_until_ready(outs)
        times.append(time.perf_counter() - t0)
    return times


def kernel(**inputs):
    inputs = {k: np.asarray(v) for k, v in inputs.items()}
    runner = _get_runner()
    maps = _prep_core_inputs(**inputs)
    out_arrs = _run_device(maps, runner)

    res = {name: np.asarray(a) for name, a in zip(runner["out_names"], out_arrs)}
    attn_all = res["attn_o"].reshape(N_CORES, NH_LOCAL, S, S)
    out_all = res["out_o"].reshape(N_CORES, S, D)

    attn = np.empty((B, 16, S, S), dtype=np.float32)
    out = np.zeros((B, S, D), dtype=np.float32)
    for c in range(N_CORES):
        b, hg = c // 4, c % 4
        attn[b, hg * NH_LOCAL:(hg + 1) * NH_LOCAL] = attn_all[c]
        out[b] += out_all[c]
    return out, attn


if __name__ == "__main__":
    rng = np.random.default_rng(0)
    sc = 1.0 / np.sqrt(D)
    ins = dict(
        v=rng.standard_normal((B, S, D), dtype=np.float32),
        k=rng.standard_normal((B, S, D), dtype=np.float32),
        q=rng.standard_normal((B, S, D), dtype=np.float32),
        wq=(rng.standard_normal((D, D)) * sc).astype(np.float32),
        bq=np.zeros(D, np.float32),
        wk=(rng.standard_normal((D, D)) * sc).astype(np.float32),
        bk=np.zeros(D, np.float32),
        wv=(rng.standard_normal((D, D)) * sc).astype(np.float32),
        bv=np.zeros(D, np.float32),
        wo=(rng.standard_normal((D, D)) * sc).astype(np.float32),
        bo=np.zeros(D, np.float32),
    )
    out, attn = kernel(**ins)
    print("out", out.shape, "attn", attn.shape)
    # quick numpy reference on a slice
    qh = (ins["q"][0] @ ins["wq"] + ins["bq"])[:, :64]
    kh = (ins["k"][0] @ ins["wk"] + ins["bk"])[:, :64]
    lg = qh @ kh.T / 8.0
    a0 = np.exp(lg - lg.max(1, keepdims=True))
    a0 /= a0.sum(1, keepdims=True)
    err = np.abs(attn[0, 0] - a0).max() / np.abs(a0).max()
    print("attn[0,0] relerr vs numpy:", err)


# revision 8
# speedup vs baseline: 1.3819x; 1.1024x over previous
"""Multi-head attention (B=2, S=2048, D=1024, H=16) on 8 TRN2 NeuronCores.

Sharding: core c = (batch b = c//4, head-group hg = c%4, 4 heads each).
Tensor-parallel over heads (wq/wk/wv column slices, wo row slices), data
parallel over batch. Host does layout prep (transposes / slices) and the
unshard (concat attn shards, sum the 4 partial outputs per batch).

All matmuls run as float32r (TF32-like, 11-bit mantissa, full PE rate;
the PE datapath rounds operands itself). Softmax skips max-subtraction
(logits ~ N(0,1), no overflow risk); the attention-probability output is
produced by recomputing logits in (q,k) orientation with normalization
fused into the exp as exp(x/8 + ln(1/sum)).
"""

import sys

if "/opt/trn_rl_repo" not in sys.path:
    sys.path.insert(0, "/opt/trn_rl_repo")

import numpy as np

B, S, D = 2, 2048, 1024
NH_LOCAL = 4          # heads per core
DEP = 64              # head depth
HD = NH_LOCAL * DEP   # 256 head-dims per core
N_CORES = 8
QS = 512              # q-stripe width
KT = 128              # k chunk (partition tile)

_CACHE = {}


def _build_program():
    import concourse.bass as bass
    import concourse.mybir as mybir
    import concourse.tile as tile
    from concourse import bacc
    from concourse.masks import make_identity

    F32 = mybir.dt.float32
    F32R = mybir.dt.float32r
    AF = mybir.ActivationFunctionType

    nc = bacc.Bacc(None, target_bir_lowering=False, debug=False)

    qT_d = nc.dram_tensor("qT", [D, S], F32R, kind="ExternalInput")
    kT_d = nc.dram_tensor("kT", [D, S], F32R, kind="ExternalInput")
    vT_d = nc.dram_tensor("vT", [D, S], F32R, kind="ExternalInput")
    wq_d = nc.dram_tensor("wq", [D, HD], F32R, kind="ExternalInput")
    wk_d = nc.dram_tensor("wk", [D, HD], F32R, kind="ExternalInput")
    wv_d = nc.dram_tensor("wv", [D, HD], F32R, kind="ExternalInput")
    bq_d = nc.dram_tensor("bq", [HD, 1], F32, kind="ExternalInput")
    bk_d = nc.dram_tensor("bk", [HD, 1], F32, kind="ExternalInput")
    bv_d = nc.dram_tensor("bv", [1, HD], F32R, kind="ExternalInput")
    wo_d = nc.dram_tensor("wo", [HD, D], F32R, kind="ExternalInput")
    bo_d = nc.dram_tensor("bo", [1, D], F32R, kind="ExternalInput")

    attn_d = nc.dram_tensor("attn_o", [NH_LOCAL, S, S], F32, kind="ExternalOutput")
    out_d = nc.dram_tensor("out_o", [S, D], F32, kind="ExternalOutput")

    n_sq = S // QS      # 4 q stripes
    n_kc = S // KT      # 16 k chunks
    n_dc = D // KT      # 8 contraction chunks for projections

    with tile.TileContext(nc) as tc:
        with (
            tc.tile_pool(name="const", bufs=1) as constp,
            tc.tile_pool(name="wpool", bufs=1) as wpool,
            tc.tile_pool(name="persist", bufs=1) as persist,
            tc.tile_pool(name="xstream", bufs=8) as xstream,
            tc.tile_pool(name="stage", bufs=4) as stage,
            tc.tile_pool(name="ptpool", bufs=10) as ptpool,
            tc.tile_pool(name="eopool", bufs=3) as eopool,
            tc.tile_pool(name="outpool", bufs=3) as outpool,
            tc.tile_pool(name="smalls", bufs=12) as smalls,
            tc.tile_pool(name="psL", bufs=3, space="PSUM") as psL,
            tc.tile_pool(name="psS", bufs=2, space="PSUM") as psS,
        ):
            # ---- constants / weights ----
            ident = constp.tile([128, 128], F32, name="ident")
            make_identity(nc, ident[:])
            ones_f = constp.tile([1, 128], F32, name="ones_f")
            nc.vector.memset(ones_f[:], 1.0)
            ones_r = constp.tile([1, 128], F32R, name="ones_r")
            nc.vector.tensor_copy(ones_r[:], ones_f[:])

            wq_t = [wpool.tile([128, HD], F32R, name=f"wq{i}") for i in range(n_dc)]
            wk_t = [wpool.tile([128, HD], F32R, name=f"wk{i}") for i in range(n_dc)]
            wv_t = [wpool.tile([128, HD], F32R, name=f"wv{i}") for i in range(n_dc)]
            for i in range(n_dc):
                nc.sync.dma_start(wq_t[i][:], wq_d[i * 128:(i + 1) * 128, :])
                nc.sync.dma_start(wk_t[i][:], wk_d[i * 128:(i + 1) * 128, :])
                nc.sync.dma_start(wv_t[i][:], wv_d[i * 128:(i + 1) * 128, :])
            wo_t = [wpool.tile([128, D], F32R, name=f"wo{i}") for i in range(2)]
            for i in range(2):
                nc.sync.dma_start(wo_t[i][:], wo_d[i * 128:(i + 1) * 128, :])
            bq_t = [wpool.tile([128, 1], F32, name=f"bq{i}") for i in range(2)]
            bk_t = [wpool.tile([128, 1], F32, name=f"bk{i}") for i in range(2)]
            for i in range(2):
                nc.sync.dma_start(bq_t[i][:], bq_d[i * 128:(i + 1) * 128, :])
                nc.sync.dma_start(bk_t[i][:], bk_d[i * 128:(i + 1) * 128, :])
            bv_t = wpool.tile([1, HD], F32R, name="bv")
            nc.sync.dma_start(bv_t[:], bv_d[:])
            bo_t = wpool.tile([1, D], F32R, name="bo")
            nc.sync.dma_start(bo_t[:], bo_d[:])

            # ---- persistent activations ----
            qhT = [persist.tile([128, S], F32R, name=f"qhT{m}") for m in range(2)]
            khT = [persist.tile([128, S], F32R, name=f"khT{m}") for m in range(2)]
            # vh_aug: per token-tile, 4 heads x (64 vals + ones col)
            vh = [persist.tile([128, NH_LOCAL * (DEP + 1)], F32R, name=f"vh{m}")
                  for m in range(n_kc)]
            ctxnT = [persist.tile([128, S], F32R, name=f"ctxnT{m}") for m in range(2)]

            ones_col = constp.tile([128, NH_LOCAL], F32, name="ones_col")
            nc.vector.memset(ones_col[:], 1.0)
            for mt in range(n_kc):
                vview = vh[mt].rearrange("p (h c) -> p h c", c=DEP + 1)
                nc.vector.tensor_copy(vview[:, :, DEP], ones_col[:])

            # ---- phase 1: projections ----
            # qhT/khT: (hd, tok) = wX.T @ xT ; vh: (tok, hd) = vT.T @ wv
            for name, xd, wt, bt, ht in (
                ("q", qT_d, wq_t, bq_t, qhT),
                ("k", kT_d, wk_t, bk_t, khT),
            ):
                for n in range(n_sq):
                    xtiles = []
                    for kk in range(n_dc):
                        xt = xstream.tile([128, QS], F32R, tag="xs",
                                          name=f"{name}x{n}_{kk}")
                        nc.sync.dma_start(
                            xt[:], xd[kk * 128:(kk + 1) * 128, n * QS:(n + 1) * QS])
                        xtiles.append(xt)
                    for m in range(2):
                        ps = psL.tile([128, QS], F32, tag="psL", name=f"ps{name}{n}{m}")
                        for kk in range(n_dc):
                            nc.tensor.matmul(
                                ps[:],
                                wt[kk][:, m * 128:(m + 1) * 128],
                                xtiles[kk][:],
                                start=(kk == 0),
                                stop=(kk == n_dc - 1),
                            )
                        nc.vector.tensor_scalar_add(
                            ht[m][:, n * QS:(n + 1) * QS], ps[:], bt[m][:])

            for n in range(n_sq):
                xtiles = []
                for kk in range(n_dc):
                    xt = xstream.tile([128, QS], F32R, tag="xs", name=f"vx{n}_{kk}")
                    nc.sync.dma_start(
                        xt[:], vT_d[kk * 128:(kk + 1) * 128, n * QS:(n + 1) * QS])
                    xtiles.append(xt)
                for j in range(QS // 128):
                    mt = n * (QS // 128) + j
                    ps = psL.tile([128, HD], F32, tag="psL", name=f"psv{mt}")
                    for kk in range(n_dc):
                        nc.tensor.matmul(
                            ps[:],
                            xtiles[kk][:, j * 128:(j + 1) * 128],
                            wv_t[kk][:],
                            start=(kk == 0),
                            stop=False,
                        )
                    nc.tensor.matmul(ps[:], ones_r[:], bv_t[:], start=False, stop=True)
                    vview = vh[mt].rearrange("p (h c) -> p h c", c=DEP + 1)
                    psview = ps.rearrange("p (h c) -> p h c", c=DEP)
                    nc.vector.tensor_copy(vview[:, :, 0:DEP], psview[:])

            # ---- phase 2: attention ----
            # Per stripe: pass A (dense logits burst + exp pairs + attnV
            # burst per head), then batched Ln (one table switch), then
            # ctx-normalize transposes, then pass B (attn output with
            # normalization fused into the exp bias), then out-proj.

            rcp_tiles = {}
            for qs in range(n_sq):
                pt_tiles = {}
                for h in range(NH_LOCAL):
                    ti, r0 = h // 2, (h % 2) * DEP
                    k_rows = khT[ti][r0:r0 + DEP, :]
                    q_rows = qhT[ti][r0:r0 + DEP, :]

                    # pass A: logits in pairs (one exp per 2 PSUM banks)
                    for kp in range(n_kc // 2):
                        pl = psL.tile([128, 2 * QS], F32, tag="psL",
                                      name=f"pl{qs}{h}{kp}")
                        for half in range(2):
                            kt = kp * 2 + half
                            nc.tensor.matmul(
                                pl[:, half * QS:(half + 1) * QS],
                                k_rows[:, kt * 128:(kt + 1) * 128],
                                q_rows[:, qs * QS:(qs + 1) * QS],
                                start=True, stop=True,
                            )
                        pt = ptpool.tile([128, 2 * QS], F32R, tag="pt",
                                         name=f"pt{qs}{h}{kp}")
                        nc.scalar.activation(
                            pt[:], pl[:], AF.Exp, bias=0.0, scale=0.125)
                        pt_tiles[(h, kp)] = pt

                    # attnV burst: ctx_aug^T accumulation over all k chunks
                    pc = psS.tile([DEP + 1, QS], F32, tag="psS", name=f"pc{qs}{h}")
                    for kt in range(n_kc):
                        pt = pt_tiles[(h, kt // 2)]
                        nc.tensor.matmul(
                            pc[:],
                            vh[kt][:, h * (DEP + 1):(h + 1) * (DEP + 1)],
                            pt[:, (kt % 2) * QS:(kt % 2 + 1) * QS],
                            start=(kt == 0),
                            stop=(kt == n_kc - 1),
                        )

                    ctxT = stage.tile([DEP + 1, QS], F32, tag="ctxT",
                                      name=f"ctxT{qs}{h}")
                    nc.vector.tensor_copy(ctxT[:], pc[:])
                    for j in range(QS // 128):
                        pt1 = psS.tile([128, DEP + 1], F32, tag="psS",
                                       name=f"pt1_{qs}{h}{j}")
                        nc.tensor.transpose(
                            pt1[:], ctxT[:, j * 128:(j + 1) * 128],
                            ident[0:DEP + 1, 0:DEP + 1])
                        rcp = smalls.tile([128, 1], F32, tag="rcp",
                                          name=f"rcp{qs}{h}{j}")
                        nc.vector.reciprocal(rcp[:], pt1[:, DEP:DEP + 1])
                        rcp_tiles[(qs, h, j)] = rcp
                        ctxn = smalls.tile([128, DEP], F32, tag="ctxn",
                                           name=f"ctxn{qs}{h}{j}")
                        nc.vector.tensor_scalar_mul(ctxn[:], pt1[:, 0:DEP], rcp[:])
                        pt2 = psS.tile([DEP, 128], F32, tag="psS",
                                       name=f"pt2_{qs}{h}{j}")
                        nc.tensor.transpose(pt2[:], ctxn[:], ident[:])
                        nc.vector.tensor_copy(
                            ctxnT[ti][r0:r0 + DEP,
                                      qs * QS + j * 128:qs * QS + (j + 1) * 128],
                            pt2[:])

                # pass B: attn output; exp unnormalized on ACT, then DVE
                # normalizes in place (keeps ACT a pure-Exp stream: one
                # table set, zero reloads)
                for h in range(NH_LOCAL):
                    ti, r0 = h // 2, (h % 2) * DEP
                    k_rows = khT[ti][r0:r0 + DEP, :]
                    q_rows = qhT[ti][r0:r0 + DEP, :]
                    for j in range(QS // 128):
                        q0 = qs * QS + j * 128
                        rcp = rcp_tiles[(qs, h, j)]
                        eo = eopool.tile([128, S], F32, tag="eo",
                                         name=f"eo{qs}{h}{j}")
                        for ksp in range(2):
                            pb = psL.tile([128, 2 * QS], F32, tag="psL",
                                          name=f"pb{qs}{h}{j}{ksp}")
                            for half in range(2):
                                ks = ksp * 2 + half
                                nc.tensor.matmul(
                                    pb[:, half * QS:(half + 1) * QS],
                                    q_rows[:, q0:q0 + 128],
                                    k_rows[:, ks * QS:(ks + 1) * QS],
                                    start=True, stop=True,
                                )
                            sl = eo[:, ksp * 2 * QS:(ksp + 1) * 2 * QS]
                            nc.scalar.activation(
                                sl, pb[:], AF.Exp, bias=0.0, scale=0.125)
                            nc.vector.tensor_scalar_mul(sl, sl, rcp[:])
                        nc.sync.dma_start(attn_d[h, q0:q0 + 128, :], eo[:])

                # ---- phase 3: output projection for this q-stripe ----
                for j in range(QS // 128):
                    q0 = qs * QS + j * 128
                    po = psL.tile([128, 2 * QS], F32, tag="psL", name=f"po{qs}{j}")
                    for n in range(2):
                        sl = po[:, n * QS:(n + 1) * QS]
                        nc.tensor.matmul(
                            sl, ctxnT[0][:, q0:q0 + 128],
                            wo_t[0][:, n * 512:(n + 1) * 512],
                            start=True, stop=False)
                        nc.tensor.matmul(
                            sl, ctxnT[1][:, q0:q0 + 128],
                            wo_t[1][:, n * 512:(n + 1) * 512],
                            start=False, stop=False)
                        nc.tensor.matmul(
                            sl, ones_r[:], bo_t[:, n * 512:(n + 1) * 512],
                            start=False, stop=True)
                    outst = outpool.tile([128, 2 * QS], F32, tag="outst",
                                         name=f"outst{qs}{j}")
                    nc.vector.tensor_copy(outst[:], po[:])
                    nc.sync.dma_start(out_d[q0:q0 + 128, :], outst[:])

    nc.finalize()
    return nc


def _get_runner():
    if "runner" in _CACHE:
        return _CACHE["runner"]

    import jax
    import jax.numpy as jnp
    from jax.sharding import Mesh, PartitionSpec, NamedSharding
    from jax.experimental.shard_map import shard_map
    import concourse.mybir as mybir
    from concourse import bass2jax

    nc = _build_program()
    bass2jax.install_neuronx_cc_hook()

    partition_name = (nc.partition_id_tensor.name
                      if nc.partition_id_tensor else None)
    in_names, out_names, out_avals = [], [], []
    for alloc in nc.m.functions[0].allocations:
        if not isinstance(alloc, mybir.MemoryLocationSet):
            continue
        name = alloc.memorylocations[0].name
        if alloc.kind == "ExternalInput":
            if name != partition_name:
                in_names.append(name)
        elif alloc.kind == "ExternalOutput":
            out_names.append(name)
            out_avals.append(
                jax.core.ShapedArray(tuple(alloc.tensor_shape),
                                     mybir.dt.np(alloc.dtype)))
    n_params = len(in_names)
    n_outs = len(out_names)
    all_in_names = in_names + out_names
    if partition_name is not None:
        all_in_names = all_in_names + [partition_name]

    def _body(*args):
        operands = list(args)
        if partition_name is not None:
            operands.append(bass2jax.partition_id_tensor())
        outs = bass2jax._bass_exec_p.bind(
            *operands,
            out_avals=tuple(out_avals),
            in_names=tuple(all_in_names),
            out_names=tuple(out_names),
            lowering_input_output_aliases=(),
            sim_require_finite=True,
            sim_require_nnan=True,
            nc=nc,
        )
        return tuple(outs)

    devices = jax.devices()[:N_CORES]
    mesh = Mesh(np.asarray(devices), ("core",))
    spec = NamedSharding(mesh, PartitionSpec("core"))
    donate = tuple(range(n_params, n_params + n_outs))
    sharded = jax.jit(
        shard_map(
            _body, mesh=mesh,
            in_specs=(PartitionSpec("core"),) * (n_params + n_outs),
            out_specs=(PartitionSpec("core"),) * n_outs,
            check_rep=False,
        ),
        donate_argnums=donate,
        keep_unused=True,
    )

    zero_shapes = [(N_CORES * a.shape[0], *a.shape[1:]) for a in out_avals]
    zero_dtypes = [a.dtype for a in out_avals]

    def _make_zeros():
        mk = jax.jit(
            lambda: tuple(
                jnp.zeros(s, d) for s, d in zip(zero_shapes, zero_dtypes)),
            out_shardings=(spec,) * n_outs)
        return mk()

    runner = {
        "nc": nc,
        "in_names": in_names,
        "out_names": out_names,
        "out_avals": out_avals,
        "sharded": sharded,
        "spec": spec,
        "make_zeros": _make_zeros,
    }
    _CACHE["runner"] = runner
    return runner


def _prep_core_inputs(v, k, q, wq, bq, wk, bk, wv, bv, wo, bo):
    """Returns per-core dict of device input arrays."""
    f = np.float32
    qT = [np.ascontiguousarray(q[b].T, dtype=f) for b in range(B)]
    kT = [np.ascontiguousarray(k[b].T, dtype=f) for b in range(B)]
    vT = [np.ascontiguousarray(v[b].T, dtype=f) for b in range(B)]
    maps = []
    for c in range(N_CORES):
        b, hg = c // 4, c % 4
        hs = slice(hg * HD, (hg + 1) * HD)
        maps.append({
            "qT": qT[b], "kT": kT[b], "vT": vT[b],
            "wq": np.ascontiguousarray(wq[:, hs], dtype=f),
            "wk": np.ascontiguousarray(wk[:, hs], dtype=f),
            "wv": np.ascontiguousarray(wv[:, hs], dtype=f),
            "bq": np.ascontiguousarray(bq[hs].reshape(HD, 1), dtype=f),
            "bk": np.ascontiguousarray(bk[hs].reshape(HD, 1), dtype=f),
            "bv": np.ascontiguousarray(bv[hs].reshape(1, HD), dtype=f),
            "wo": np.ascontiguousarray(wo[hs, :], dtype=f),
            "bo": (np.ascontiguousarray(bo.reshape(1, D), dtype=f)
                   if hg == 0 else np.zeros((1, D), dtype=f)),
        })
    return maps


def _run_device(maps, runner, reps=1):
    import jax

    concat = [
        jax.device_put(
            np.concatenate([maps[c][name] for c in range(N_CORES)], axis=0),
            runner["spec"])
        for name in runner["in_names"]
    ]
    out = None
    for _ in range(reps):
        zeros = runner["make_zeros"]()
        out = runner["sharded"](*concat, *zeros)
    return out


def benchmark(inputs, reps=20):
    """Times device execution (inputs resident, outputs ping-pong donated).

    Returns list of per-rep wall seconds.
    """
    import time
    import jax

    inputs = {k: np.asarray(v) for k, v in inputs.items()}
    runner = _get_runner()
    maps = _prep_core_inputs(**inputs)
    concat = [
        jax.device_put(
            np.concatenate([maps[c][name] for c in range(N_CORES)], axis=0),
            runner["spec"])
        for name in runner["in_names"]
    ]
    outs = runner["sharded"](*concat, *runner["make_zeros"]())
    jax.block_until_ready(outs)
    times = []
    for _ in range(reps):
        t0 = time.perf_counter()
        outs = runner["sharded"](*concat, *outs)
        jax.block_until_ready(outs)
        times.append(time.perf_counter() - t0)
    return times


def kernel(**inputs):
    inputs = {k: np.asarray(v) for k, v in inputs.items()}
    runner = _get_runner()
    maps = _prep_core_inputs(**inputs)
    out_arrs = _run_device(maps, runner)

    res = {name: np.asarray(a) for name, a in zip(runner["out_names"], out_arrs)}
    attn_all = res["attn_o"].reshape(N_CORES, NH_LOCAL, S, S)
    out_all = res["out_o"].reshape(N_CORES, S, D)

    attn = np.empty((B, 16, S, S), dtype=np.float32)
    out = np.zeros((B, S, D), dtype=np.float32)
    for c in range(N_CORES):
        b, hg = c // 4, c % 4
        attn[b, hg * NH_LOCAL:(hg + 1) * NH_LOCAL] = attn_all[c]
        out[b] += out_all[c]
    return out, attn


if __name__ == "__main__":
    rng = np.random.default_rng(0)
    sc = 1.0 / np.sqrt(D)
    ins = dict(
        v=rng.standard_normal((B, S, D), dtype=np.float32),
        k=rng.standard_normal((B, S, D), dtype=np.float32),
        q=rng.standard_normal((B, S, D), dtype=np.float32),
        wq=(rng.standard_normal((D, D)) * sc).astype(np.float32),
        bq=np.zeros(D, np.float32),
        wk=(rng.standard_normal((D, D)) * sc).astype(np.float32),
        bk=np.zeros(D, np.float32),
        wv=(rng.standard_normal((D, D)) * sc).astype(np.float32),
        bv=np.zeros(D, np.float32),
        wo=(rng.standard_normal((D, D)) * sc).astype(np.float32),
        bo=np.zeros(D, np.float32),
    )
    out, attn = kernel(**ins)
    print("out", out.shape, "attn", attn.shape)
    # quick numpy reference on a slice
    qh = (ins["q"][0] @ ins["wq"] + ins["bq"])[:, :64]
    kh = (ins["k"][0] @ ins["wk"] + ins["bk"])[:, :64]
    lg = qh @ kh.T / 8.0
    a0 = np.exp(lg - lg.max(1, keepdims=True))
    a0 /= a0.sum(1, keepdims=True)
    err = np.abs(attn[0, 0] - a0).max() / np.abs(a0).max()
    print("attn[0,0] relerr vs numpy:", err)
